# revision 2
# baseline (speedup 1.0000x reference)
"""Trainium2 Bass kernel for the CLIP text/image concat multi-head classifier.

Full (unsharded) inputs in, full outputs out. The 312 heads are sharded
39-per-core across 8 NeuronCores (head parallel); outputs are gathered and
concatenated along the class axis on the host. No collectives.

v2 design (vs the 104-row baseline):
  - All text-side terms are folded on the host: per-(head,hidden) text dot
    products become bias constants, logit norms and exp(logit_scale) are
    folded into pre-normalized fp16 operands. The device only runs
    batch-dependent GEMMs.
  - Classifier rows (head,hidden) are padded to 12288 = 96 tiles x 128 rows
    so every image matmul uses the full 128-wide PE partition dim.
  - Per tile only 3 small-engine instrs remain (relu + bn_stats + bn_aggr);
    batchnorm scale/shift computation is batched over all 96 tiles with a
    handful of strided instructions at the end, then folded into a
    block-diagonal projection matmul (At) accumulated over all tiles.
  - Weights stream in groups of 4 tiles per DMA, classifier1 on the SP
    HWDGE queue and classifier2 on the ACT HWDGE queue, interleaved so both
    queues run concurrently through the whole z-pass.
"""

import os
import sys
from contextlib import ExitStack

for _p in ("/opt/trn_rl_repo", "/root/.axon_site/_ro/trn_rl_repo"):
    if os.path.isdir(_p) and _p not in sys.path:
        sys.path.insert(0, _p)

import numpy as np
import concourse.bass as bass
import concourse.tile as tile
from concourse import bacc, mybir
from concourse.bass_utils import run_bass_kernel_spmd

F32 = mybir.dt.float32
F16 = mybir.dt.float16
AF = mybir.ActivationFunctionType
ts = bass.ts

B, N, DE, DV, H = 256, 312, 512, 768, 312
EPS = 1e-5
NC = 8
NH = N // NC              # 39 heads per core
ROWS = NH * H             # 12168 true (head, hidden) rows per core
TR = 128                  # rows per tile
NT = 96                   # tiles per classifier (12288 padded rows)
PADROWS = NT * TR         # 12288
G = 4                     # tiles per weight DMA group
NG = NT // G              # 24 weight DMAs per classifier
C1D = DE // 128           # 4 contraction chunks (classifier1 / lin1 / logits)
C2D = DV // 128           # 6 contraction chunks (classifier2 / lin2)
RC = 258                  # R row-tile stride: 256 batch + mean col + pad


def _emit_body(nc, tc, ctx, ins, outs):
    const = ctx.enter_context(tc.tile_pool(name="const", bufs=1))
    sp = ctx.enter_context(tc.tile_pool(name="sp", bufs=4))

    def ld(name, shape, dt):
        t = const.tile(shape, dt, tag=name)
        nc.sync.dma_start(t[:], ins[name][:])
        return t

    imgT = ld("imgT", [128, C1D * B], F16)
    w1iT = ld("w1iT", [128, C1D * NH], F16)
    ioutT = ld("ioutT", [128, C2D * B], F16)
    w2iT = ld("w2iT", [128, C2D * NH], F16)
    textNT = ld("textNT", [128, C1D * NH], F16)
    imgNT = ld("imgNT", [128, C1D * B], F16)
    lb1 = ld("lb1", [NH, 1], F32)
    lb2 = ld("lb2", [NH, 1], F32)
    bc1 = ld("bc1", [128, NT], F32)
    bc2 = ld("bc2", [128, NT], F32)
    w2g1 = ld("w2g1", [128, NT], F32)
    w2g2 = ld("w2g2", [128, NT], F32)
    cst1 = ld("cst1", [NH, 1], F32)
    cst2 = ld("cst2", [NH, 1], F32)
    htall = ld("ht", [128, NT * NH], F16)

    R1 = const.tile([128, NT * RC], F16, tag="R1")
    R2 = const.tile([128, NT * RC], F16, tag="R2")
    SV1 = const.tile([128, 2 * NT], F32, tag="SV1")
    SV2 = const.tile([128, 2 * NT], F32, tag="SV2")
    AT1 = const.tile([128, NT * NH], F16, tag="AT1")
    AT2 = const.tile([128, NT * NH], F16, tag="AT2")
    eps_col = const.tile([128, 1], F32, tag="eps_col")
    nc.vector.memset(eps_col[:], EPS)

    # ---- phase A: lin1 / lin2 / logits (all postprocessing host-folded) ----
    with tc.tile_pool(name="spp", bufs=3, space="PSUM") as spp:
        for (wT, imt, nch, lbt, oname) in (
                (w1iT, imgT, C1D, lb1, "lin1_o"),
                (w2iT, ioutT, C2D, lb2, "lin2_o")):
            lp = spp.tile([NH, B], F32, tag="lp")
            for ch in range(nch):
                nc.tensor.matmul(lp[:], wT[:, ts(ch, NH)], imt[:, ts(ch, B)],
                                 start=(ch == 0), stop=(ch == nch - 1))
            lsb = sp.tile([NH, B], F32, tag="lsb")
            nc.scalar.activation(lsb[:], lp[:], AF.Identity, bias=lbt[:])
            nc.sync.dma_start(outs[oname][:], lsb[:])
        lg = spp.tile([NH, B], F32, tag="lp")
        for ch in range(C1D):
            nc.tensor.matmul(lg[:], textNT[:, ts(ch, NH)], imgNT[:, ts(ch, B)],
                             start=(ch == 0), stop=(ch == C1D - 1))
        lgs = sp.tile([NH, B], F32, tag="lsb")
        nc.scalar.copy(lgs[:], lg[:])
        nc.sync.dma_start(outs["lgt_o"][:], lgs[:])

    # ---- classifiers: z-pass (image matmuls + relu + stats) ----
    with tc.tile_pool(name="wp1", bufs=3) as wp1, \
         tc.tile_pool(name="wp2", bufs=3) as wp2, \
         tc.tile_pool(name="zp", bufs=6, space="PSUM") as zp, \
         tc.tile_pool(name="pp", bufs=2, space="PSUM") as pp:
        sv31 = SV1[:].rearrange("p (a t) -> p a t", a=2)
        sv32 = SV2[:].rearrange("p (a t) -> p a t", a=2)
        cfg = ((ins["wm1"], C1D, imgT, bc1, R1, sv31, wp1, nc.sync),
               (ins["wm2"], C2D, ioutT, bc2, R2, sv32, wp2, nc.scalar))
        for grp in range(NG):
            for ci, (wm_in, nch, imt, bct, Rt, sv3, wp, dmaeng) in enumerate(cfg):
                wm = wp.tile([128, G * nch * 128], F16, tag="wm")
                dmaeng.dma_start(wm[:], wm_in[grp])
                for g in range(G):
                    t = grp * G + g
                    z = zp.tile([128, B], F32, tag="z")
                    for ch in range(nch):
                        nc.tensor.matmul(z[:], wm[:, ts(g * nch + ch, 128)],
                                         imt[:, ts(ch, B)],
                                         start=(ch == 0), stop=(ch == nch - 1))
                    nc.scalar.activation(Rt[:, RC * t:RC * t + B], z[:],
                                         AF.Relu, bias=bct[:, t:t + 1])
                    st6 = sp.tile([128, 6], F32, tag="st6")
                    nc.vector.bn_stats(st6[:], Rt[:, RC * t:RC * t + B])
                    nc.vector.bn_aggr(sv3[:, :, t:t + 1], st6[:])

        # ---- batched BN fold + block-diagonal projection ----
        for (Rt, SVt, ATt, w2gt, cstt, oname) in (
                (R1, SV1, AT1, w2g1, cst1, "cls1_o"),
                (R2, SV2, AT2, w2g2, cst2, "cls2_o")):
            sq = sp.tile([128, NT], F32, tag="sq")
            nc.scalar.activation(sq[:], SVt[:, NT:2 * NT], AF.Sqrt,
                                 bias=eps_col[:])
            inv = sp.tile([128, NT], F32, tag="inv")
            nc.vector.reciprocal(inv[:], sq[:])
            am = sp.tile([128, NT], F16, tag="am")
            nc.vector.tensor_mul(am[:], inv[:], w2gt[:])
            r3 = Rt[:].rearrange("p (t c) -> p t c", c=RC)
            nc.scalar.copy(r3[:, :, B:B + 1], SVt[:, 0:NT].unsqueeze(2))
            at3 = ATt[:].rearrange("p (t j) -> p t j", j=NH)
            ht3 = htall[:].rearrange("p (t j) -> p t j", j=NH)
            am3 = am[:].unsqueeze(2).broadcast_to((128, NT, NH))
            nc.vector.tensor_mul(at3, ht3, am3)
            ppt = pp.tile([NH, B + 1], F32, tag="ppt")
            for t in range(NT):
                nc.tensor.matmul(ppt[:], ATt[:, ts(t, NH)],
                                 Rt[:, RC * t:RC * t + B + 1],
                                 start=(t == 0), stop=(t == NT - 1))
            mcol = sp.tile([NH, 1], F32, tag="mcol")
            nc.vector.tensor_sub(mcol[:], cstt[:], ppt[:, B:B + 1])
            csb = sp.tile([NH, B], F32, tag="csb")
            nc.vector.tensor_scalar_add(csb[:], ppt[:, :B], mcol[:])
            nc.sync.dma_start(outs[oname][:], csb[:])


def _build(loop_k=1):
    nc = bacc.Bacc("TRN2", target_bir_lowering=False, debug=False,
                   num_devices=NC)
    mk = nc.dram_tensor

    def inp(name, shape, dt):
        return mk(name, shape, dt, kind="ExternalInput").ap()

    ins = {
        "imgT": inp("imgT", [128, C1D * B], F16),
        "ioutT": inp("ioutT", [128, C2D * B], F16),
        "imgNT": inp("imgNT", [128, C1D * B], F16),
        "textNT": inp("textNT", [128, C1D * NH], F16),
        "w1iT": inp("w1iT", [128, C1D * NH], F16),
        "w2iT": inp("w2iT", [128, C2D * NH], F16),
        "lb1": inp("lb1", [NH, 1], F32),
        "lb2": inp("lb2", [NH, 1], F32),
        "bc1": inp("bc1", [128, NT], F32),
        "bc2": inp("bc2", [128, NT], F32),
        "w2g1": inp("w2g1", [128, NT], F32),
        "w2g2": inp("w2g2", [128, NT], F32),
        "cst1": inp("cst1", [NH, 1], F32),
        "cst2": inp("cst2", [NH, 1], F32),
        "ht": inp("ht", [128, NT * NH], F16),
        "wm1": inp("wm1", [NG, 128, G * C1D * 128], F16),
        "wm2": inp("wm2", [NG, 128, G * C2D * 128], F16),
    }
    outs = {
        k: mk(k, [NH, B], F32, kind="ExternalOutput").ap()
        for k in ("lin1_o", "lin2_o", "cls1_o", "cls2_o", "lgt_o")
    }

    with tile.TileContext(nc) as tc:
        with ExitStack() as ctx:
            if loop_k > 1:
                with tc.For_i(0, loop_k, 1):
                    _emit_body(nc, tc, ctx, ins, outs)
            else:
                _emit_body(nc, tc, ctx, ins, outs)
    nc.compile()
    return nc


def _pack_T(x, nch, dtype=np.float16):
    # x: [rows, d=nch*128] -> [128, nch*rows]: el [p, ch*rows + r] = x[r, ch*128+p]
    rows = x.shape[0]
    return np.ascontiguousarray(
        x.T.reshape(nch, 128, rows).transpose(1, 0, 2).reshape(128, nch * rows)
    ).astype(dtype)


def _pack_w_groups(w, nch):
    # w: [PADROWS, nch*128] -> [NG, 128, G*nch*128]
    # el [grp, p, (g*nch+ch)*128 + r] = w[(grp*G+g)*128 + r, ch*128 + p]
    return np.ascontiguousarray(
        w.reshape(NG, G, 128, nch, 128).transpose(0, 4, 1, 3, 2)
        .reshape(NG, 128, G * nch * 128)
    ).astype(np.float16)


def _pack_cols(v):
    # v: [PADROWS] -> [128, NT], column t = v[t*128:(t+1)*128]
    return np.ascontiguousarray(v.reshape(NT, TR).T.astype(np.float32))


def _padrows(x):
    # x: [ROWS, ...] -> [PADROWS, ...] zero-padded
    pad = np.zeros((PADROWS - ROWS,) + x.shape[1:], x.dtype)
    return np.concatenate([x, pad], axis=0)


def host_prep(inputs):
    f32 = np.float32
    g = {k: np.asarray(v, f32) for k, v in inputs.items()}
    image_embed, text_embed = g["image_embed"], g["text_embed"]
    image_out, text_out = g["image_out"], g["text_out"]

    imgT = _pack_T(image_embed, C1D)
    ioutT = _pack_T(image_out, C2D)
    imgN = image_embed / np.linalg.norm(image_embed, axis=1, keepdims=True)
    imgNT = _pack_T(imgN, C1D)
    scale_t = np.exp(g["logit_scale"]) / np.linalg.norm(text_embed, axis=1)

    # head -> within-core head index map for padded rows
    ht_full = np.zeros((PADROWS, NH), np.float16)
    rows = np.arange(ROWS)
    ht_full[rows, rows // H] = 1.0
    ht = np.ascontiguousarray(
        ht_full.reshape(NT, TR, NH).transpose(1, 0, 2).reshape(128, NT * NH))

    in_maps = []
    for c in range(NC):
        S = slice(c * NH, (c + 1) * NH)
        w1 = _pack_w_groups(_padrows(g["C1_W1"][S, :, :DE].reshape(ROWS, DE)), C1D)
        w2 = _pack_w_groups(_padrows(g["C2_W1"][S, :, :DV].reshape(ROWS, DV)), C2D)

        # text-side folds
        lb1 = g["b1"][S] + np.einsum("nd,nd->n", text_embed[S], g["W1"][S, DE:])
        lb2 = g["b2"][S] + np.einsum("nd,nd->n", text_out[S], g["W2"][S, DV:])
        bc1 = g["C1_b1"][S] + np.einsum("nd,nhd->nh", text_embed[S],
                                        g["C1_W1"][S, :, DE:])
        bc2 = g["C2_b1"][S] + np.einsum("nd,nhd->nh", text_out[S],
                                        g["C2_W1"][S, :, DV:])
        w2gam1 = (g["C1_W2"][S] * g["C1_gamma"][S]).reshape(ROWS)
        w2gam2 = (g["C2_W2"][S] * g["C2_gamma"][S]).reshape(ROWS)
        cst1 = g["C1_b2"][S] + (g["C1_W2"][S] * g["C1_beta"][S]).sum(1)
        cst2 = g["C2_b2"][S] + (g["C2_W2"][S] * g["C2_beta"][S]).sum(1)
        textN = text_embed[S] * scale_t[S][:, None]

        in_maps.append({
            "imgT": imgT, "ioutT": ioutT, "imgNT": imgNT,
            "textNT": _pack_T(textN, C1D),
            "w1iT": _pack_T(g["W1"][S, :DE], C1D),
            "w2iT": _pack_T(g["W2"][S, :DV], C2D),
            "lb1": np.ascontiguousarray(lb1[:, None], f32),
            "lb2": np.ascontiguousarray(lb2[:, None], f32),
            "bc1": _pack_cols(_padrows(bc1.reshape(ROWS))),
            "bc2": _pack_cols(_padrows(bc2.reshape(ROWS))),
            "w2g1": _pack_cols(_padrows(w2gam1)),
            "w2g2": _pack_cols(_padrows(w2gam2)),
            "cst1": np.ascontiguousarray(cst1[:, None], f32),
            "cst2": np.ascontiguousarray(cst2[:, None], f32),
            "ht": ht,
            "wm1": w1, "wm2": w2,
        })
    return in_maps


_cache = {}


def _get_nc(loop_k=1):
    if loop_k not in _cache:
        _cache[loop_k] = _build(loop_k)
    return _cache[loop_k]


def run(inputs, loop_k=1):
    nc = _get_nc(loop_k)
    in_maps = host_prep(inputs)
    res = run_bass_kernel_spmd(nc, in_maps, core_ids=list(range(NC)))
    names = ("lin1_o", "lin2_o", "cls1_o", "cls2_o", "lgt_o")
    full = []
    for nm in names:
        parts = [res.results[c][nm] for c in range(NC)]
        full.append(np.ascontiguousarray(np.concatenate(parts, axis=0).T))
    return tuple(full)


def kernel(**inputs):
    return run(inputs, loop_k=1)
